# revision 4
# baseline (speedup 1.0000x reference)
"""Attention-gate kernel for Trainium2 (8 NeuronCores, batch-parallel).

Computation (per reference):
    g   = BN_g(input_g @ Wg + bg)          # 1x1 conv 256->128 + BN
    x   = BN_x(input_x @ Wx + bx)          # 1x1 conv 256->128 + BN
    s   = relu(g + x)
    psi = sigmoid(BN_p(s @ Wp + bp))       # 128->1 conv + BN + sigmoid
    out = x * psi                          # [8,128,128,128] f32

Strategy:
  - Shard batch across the 8 cores (core b <- batch b).
  - Host folds BN+bias into the conv weights (W' = W * A, per-output-channel
    scale) and per-channel bias vectors, and pre-transposes the activations
    to channels-first [256, 16384] so the device sees pure contiguous DMA +
    channel-contraction matmuls.
  - Device (per core, 32 blocks x 512 pixels):
      P_g = Wg'^T Xg       (2 matmuls, float32r - fast path; error is
                            attenuated through the sigmoid gate)
      P_x = Wx'^T Xx       (2 matmuls, float32  - exact, direct output factor)
      u   = P_g + P_x                        (DVE)
      s   = relu(u + D_s)                    (ACT, -> float32r)
      P_p = wpo^T s        (1 matmul, float32r; wpo = Wp*A_p replicated to
                            128 columns => result broadcast to all partitions)
      psi = sigmoid(P_p + d_p)               (ACT)
      xv  = P_x + D_x                        (DVE)
      o   = xv * psi                         (DVE)
  - Output is produced channels-first [128, 16384] per core; host transposes
    back. All DMA is >= 2KB-contiguous per partition row.
"""
import numpy as np

B, H, W, CIN, F = 8, 128, 128, 256, 128
NPIX = H * W          # 16384 pixels per batch/core
NBLK = 512            # pixels per block (one PSUM bank at f32)
NB = NPIX // NBLK     # 32 blocks
NCORES = 8
EPS = 1e-3

_CACHE = {}


def _build_program():
    import sys
    if "/opt/trn_rl_repo" not in sys.path:
        sys.path.insert(0, "/opt/trn_rl_repo")
    from contextlib import ExitStack
    import concourse.bacc as bacc
    import concourse.tile as tile
    from concourse import mybir

    f32 = mybir.dt.float32
    f32r = mybir.dt.float32r
    AF = mybir.ActivationFunctionType

    nc = bacc.Bacc("TRN2", target_bir_lowering=False, debug=False,
                   enable_asserts=False)
    xg = nc.dram_tensor("xg", [CIN, NPIX], f32r, kind="ExternalInput").ap()
    xx = nc.dram_tensor("xx", [CIN, NPIX], f32, kind="ExternalInput").ap()
    wg = nc.dram_tensor("wg", [CIN, F], f32r, kind="ExternalInput").ap()
    wx = nc.dram_tensor("wx", [CIN, F], f32, kind="ExternalInput").ap()
    wpo = nc.dram_tensor("wpo", [F, 128], f32r, kind="ExternalInput").ap()
    dvec = nc.dram_tensor("dvec", [128, 3], f32, kind="ExternalInput").ap()
    out_t = nc.dram_tensor("out_t", [F, NPIX], f32, kind="ExternalOutput").ap()

    with tile.TileContext(nc) as tc:
        with ExitStack() as ctx:
            consts = ctx.enter_context(tc.tile_pool(name="consts", bufs=1))
            io_in = ctx.enter_context(tc.tile_pool(name="io_in", bufs=6))
            work = ctx.enter_context(tc.tile_pool(name="work", bufs=3))
            io_out = ctx.enter_context(tc.tile_pool(name="io_out", bufs=4))
            px_pool = ctx.enter_context(tc.tile_pool(name="px", bufs=3, space="PSUM"))
            pp_pool = ctx.enter_context(tc.tile_pool(name="pp", bufs=3, space="PSUM"))

            wg0 = consts.tile([128, F], f32r)
            nc.sync.dma_start(out=wg0, in_=wg[0:128, :])
            wg1 = consts.tile([128, F], f32r)
            nc.sync.dma_start(out=wg1, in_=wg[128:256, :])
            wx0 = consts.tile([128, F], f32)
            nc.sync.dma_start(out=wx0, in_=wx[0:128, :])
            wx1 = consts.tile([128, F], f32)
            nc.sync.dma_start(out=wx1, in_=wx[128:256, :])
            wpo_sb = consts.tile([F, 128], f32r)
            nc.sync.dma_start(out=wpo_sb, in_=wpo)
            dv = consts.tile([128, 3], f32)
            nc.sync.dma_start(out=dv, in_=dvec)
            d_s = dv[:, 0:1]
            d_x = dv[:, 1:2]
            d_p = dv[:, 2:3]

            for i in range(NB):
                sl = slice(i * NBLK, (i + 1) * NBLK)
                xg0 = io_in.tile([128, NBLK], f32r, name=f"xg0_{i}", tag="xg0")
                nc.sync.dma_start(out=xg0, in_=xg[0:128, sl])
                xg1 = io_in.tile([128, NBLK], f32r, name=f"xg1_{i}", tag="xg1")
                nc.sync.dma_start(out=xg1, in_=xg[128:256, sl])
                xx0 = io_in.tile([128, NBLK], f32, name=f"xx0_{i}", tag="xx0")
                nc.sync.dma_start(out=xx0, in_=xx[0:128, sl])
                xx1 = io_in.tile([128, NBLK], f32, name=f"xx1_{i}", tag="xx1")
                nc.sync.dma_start(out=xx1, in_=xx[128:256, sl])

                # P_x = Wx'^T Xx (fp32, exact)
                ps_x = px_pool.tile([128, NBLK], f32, name=f"ps_x_{i}", tag="ps_x")
                nc.tensor.matmul(ps_x, wx0, xx0, start=True, stop=False)
                nc.tensor.matmul(ps_x, wx1, xx1, start=False, stop=True)

                # xv = P_x + D_x -> SBUF (frees the bank value for reuse)
                xv = work.tile([128, NBLK], f32, name=f"xv_{i}", tag="xv")
                nc.vector.tensor_scalar_add(xv, ps_x, d_x)

                # accumulate the g-branch (f32r) onto the same bank:
                # bank becomes P_s = P_g + P_x  (WAR on xv read; Tile serializes)
                nc.tensor.matmul(ps_x, wg0, xg0, start=False, stop=False,
                                 skip_group_check=True)
                nc.tensor.matmul(ps_x, wg1, xg1, start=False, stop=True,
                                 skip_group_check=True)

                s_sb = work.tile([128, NBLK], f32r, name=f"s_{i}", tag="s")
                nc.scalar.activation(s_sb, ps_x, AF.Relu, bias=d_s, scale=1.0)

                ps_p = pp_pool.tile([128, NBLK], f32, name=f"ps_p_{i}", tag="ps_p")
                nc.tensor.matmul(ps_p, wpo_sb, s_sb, start=True, stop=True)

                psi = work.tile([128, NBLK], f32, name=f"psi_{i}", tag="psi")
                nc.scalar.activation(psi, ps_p, AF.Sigmoid, bias=d_p, scale=1.0)

                o_sb = io_out.tile([128, NBLK], f32, name=f"o_{i}", tag="o")
                nc.vector.tensor_mul(o_sb, xv, psi)

                nc.sync.dma_start(out=out_t[:, sl], in_=o_sb)

    nc.compile()
    return nc


def _get_program():
    if "nc" not in _CACHE:
        _CACHE["nc"] = _build_program()
    return _CACHE["nc"]


def kernel(input_g, input_x, Wg, bg, gamma_g, beta_g, mean_g, var_g,
           Wx, bx, gamma_x, beta_x, mean_x, var_x,
           Wp, bp, gamma_p, beta_p, mean_p, var_p):
    import sys
    if "/opt/trn_rl_repo" not in sys.path:
        sys.path.insert(0, "/opt/trn_rl_repo")
    from concourse import bass_utils

    nc = _get_program()

    # Fold BN (+conv bias) into weights/biases on host, in float64 for accuracy.
    f8 = np.float64
    A_g = (gamma_g.astype(f8) / np.sqrt(var_g.astype(f8) + EPS))
    C_g = beta_g.astype(f8) - mean_g.astype(f8) * A_g + bg.astype(f8) * A_g
    A_x = (gamma_x.astype(f8) / np.sqrt(var_x.astype(f8) + EPS))
    C_x = beta_x.astype(f8) - mean_x.astype(f8) * A_x + bx.astype(f8) * A_x
    A_p = (gamma_p.astype(f8) / np.sqrt(var_p.astype(f8) + EPS))[0]
    C_p = (beta_p.astype(f8) - mean_p.astype(f8) * A_p)[0]

    wg_eff = (Wg.astype(f8) * A_g[None, :]).astype(np.float32)
    wx_eff = (Wx.astype(f8) * A_x[None, :]).astype(np.float32)
    wpo = np.ascontiguousarray(
        np.repeat((Wp[:, 0].astype(f8) * A_p)[:, None], 128, axis=1)
    ).astype(np.float32)
    d_s = (C_g + C_x).astype(np.float32)
    d_x = C_x.astype(np.float32)
    d_p = np.full((128,), A_p * bp.astype(f8)[0] + C_p, dtype=np.float32)
    dvec = np.ascontiguousarray(np.stack([d_s, d_x, d_p], axis=1))

    in_maps = []
    for b in range(NCORES):
        xg_t = np.ascontiguousarray(input_g[b].reshape(NPIX, CIN).T)
        xx_t = np.ascontiguousarray(input_x[b].reshape(NPIX, CIN).T)
        in_maps.append(dict(xg=xg_t, xx=xx_t, wg=wg_eff, wx=wx_eff,
                            wpo=wpo, dvec=dvec))

    res = bass_utils.run_bass_kernel_spmd(nc, in_maps,
                                          core_ids=list(range(NCORES)))
    _CACHE["last_results"] = res

    out = np.empty((B, H, W, F), np.float32)
    for b in range(NCORES):
        out[b] = res.results[b]["out_t"].T.reshape(H, W, F)
    return out


# revision 6
# speedup vs baseline: 1.2154x; 1.2154x over previous
"""Attention-gate kernel for Trainium2 (8 NeuronCores, batch-parallel).

Computation (per reference):
    g   = BN_g(input_g @ Wg + bg)          # 1x1 conv 256->128 + BN
    x   = BN_x(input_x @ Wx + bx)          # 1x1 conv 256->128 + BN
    s   = relu(g + x)
    psi = sigmoid(BN_p(s @ Wp + bp))       # 128->1 conv + BN + sigmoid
    out = x * psi                          # [8,128,128,128] f32

Strategy:
  - Shard batch across the 8 cores (core b <- batch b).
  - Host folds BN+bias into the conv weights (W' = W * A, per-output-channel
    scale) and per-channel bias vectors, and pre-transposes the activations
    to channels-first [256, 16384] so the device sees pure contiguous DMA +
    channel-contraction matmuls.
  - Device (per core, 32 blocks x 512 pixels):
      P_g = Wg'^T Xg       (2 matmuls, float32r - fast path; error is
                            attenuated through the sigmoid gate)
      P_x = Wx'^T Xx       (2 matmuls, float32  - exact, direct output factor)
      u   = P_g + P_x                        (DVE)
      s   = relu(u + D_s)                    (ACT, -> float32r)
      P_p = wpo^T s        (1 matmul, float32r; wpo = Wp*A_p replicated to
                            128 columns => result broadcast to all partitions)
      psi = sigmoid(P_p + d_p)               (ACT)
      xv  = P_x + D_x                        (DVE)
      o   = xv * psi                         (DVE)
  - Output is produced channels-first [128, 16384] per core; host transposes
    back. All DMA is >= 2KB-contiguous per partition row.
"""
import numpy as np

B, H, W, CIN, F = 8, 128, 128, 256, 128
NPIX = H * W          # 16384 pixels per batch/core
NBLK = 512            # pixels per block (one PSUM bank at f32)
NB = NPIX // NBLK     # 32 blocks
NCORES = 8
EPS = 1e-3

_CACHE = {}


def _build_program():
    import sys
    if "/opt/trn_rl_repo" not in sys.path:
        sys.path.insert(0, "/opt/trn_rl_repo")
    from contextlib import ExitStack
    import concourse.bacc as bacc
    import concourse.tile as tile
    from concourse import mybir

    f32 = mybir.dt.float32
    f32r = mybir.dt.float32r
    AF = mybir.ActivationFunctionType

    nc = bacc.Bacc("TRN2", target_bir_lowering=False, debug=False,
                   enable_asserts=False)
    xg = nc.dram_tensor("xg", [CIN, NPIX], f32r, kind="ExternalInput").ap()
    xx = nc.dram_tensor("xx", [CIN, NPIX], f32, kind="ExternalInput").ap()
    wg = nc.dram_tensor("wg", [CIN, F], f32r, kind="ExternalInput").ap()
    wx = nc.dram_tensor("wx", [CIN, F], f32, kind="ExternalInput").ap()
    wpo = nc.dram_tensor("wpo", [F, 128], f32r, kind="ExternalInput").ap()
    dvec = nc.dram_tensor("dvec", [128, 3], f32, kind="ExternalInput").ap()
    out_t = nc.dram_tensor("out_t", [F, NPIX], f32, kind="ExternalOutput").ap()

    with tile.TileContext(nc) as tc:
        with ExitStack() as ctx:
            consts = ctx.enter_context(tc.tile_pool(name="consts", bufs=1))
            io_in = ctx.enter_context(tc.tile_pool(name="io_in", bufs=2))
            work = ctx.enter_context(tc.tile_pool(name="work", bufs=3))
            io_out = ctx.enter_context(tc.tile_pool(name="io_out", bufs=2))
            px_pool = ctx.enter_context(tc.tile_pool(name="px", bufs=3, space="PSUM"))
            pp_pool = ctx.enter_context(tc.tile_pool(name="pp", bufs=3, space="PSUM"))

            wg0 = consts.tile([128, F], f32r)
            nc.sync.dma_start(out=wg0, in_=wg[0:128, :])
            wg1 = consts.tile([128, F], f32r)
            nc.sync.dma_start(out=wg1, in_=wg[128:256, :])
            wx0 = consts.tile([128, F], f32)
            nc.sync.dma_start(out=wx0, in_=wx[0:128, :])
            wx1 = consts.tile([128, F], f32)
            nc.sync.dma_start(out=wx1, in_=wx[128:256, :])
            wpo_sb = consts.tile([F, 128], f32r)
            nc.sync.dma_start(out=wpo_sb, in_=wpo)
            dv = consts.tile([128, 3], f32)
            nc.sync.dma_start(out=dv, in_=dvec)
            d_s = dv[:, 0:1]
            d_x = dv[:, 1:2]
            d_p = dv[:, 2:3]

            GB = 4                # sub-blocks per DMA group
            GBLK = GB * NBLK      # 2048 pixels per DMA (8KB rows)
            for g in range(NB // GB):
                gsl = slice(g * GBLK, (g + 1) * GBLK)
                xg_b0 = io_in.tile([128, GBLK], f32r, name=f"xgb0_{g}", tag="xgb0")
                nc.sync.dma_start(out=xg_b0, in_=xg[0:128, gsl])
                xg_b1 = io_in.tile([128, GBLK], f32r, name=f"xgb1_{g}", tag="xgb1")
                nc.sync.dma_start(out=xg_b1, in_=xg[128:256, gsl])
                xx_b0 = io_in.tile([128, GBLK], f32, name=f"xxb0_{g}", tag="xxb0")
                nc.sync.dma_start(out=xx_b0, in_=xx[0:128, gsl])
                xx_b1 = io_in.tile([128, GBLK], f32, name=f"xxb1_{g}", tag="xxb1")
                nc.sync.dma_start(out=xx_b1, in_=xx[128:256, gsl])

                o_big = io_out.tile([128, GBLK], f32, name=f"ob_{g}", tag="ob")

                for j in range(GB):
                    i = g * GB + j
                    sl = slice(j * NBLK, (j + 1) * NBLK)

                    # P_x = Wx'^T Xx (fp32, exact)
                    ps_x = px_pool.tile([128, NBLK], f32, name=f"ps_x_{i}", tag="ps_x")
                    nc.tensor.matmul(ps_x, wx0, xx_b0[:, sl], start=True, stop=False)
                    nc.tensor.matmul(ps_x, wx1, xx_b1[:, sl], start=False, stop=True)

                    # xv = P_x + D_x -> SBUF (frees the bank value for reuse)
                    xv = work.tile([128, NBLK], f32, name=f"xv_{i}", tag="xv")
                    nc.vector.tensor_scalar_add(xv, ps_x, d_x)

                    # accumulate the g-branch (f32r) onto the same bank:
                    # bank becomes P_s = P_g + P_x (WAR on xv read; Tile serializes)
                    nc.tensor.matmul(ps_x, wg0, xg_b0[:, sl], start=False, stop=False,
                                     skip_group_check=True)
                    nc.tensor.matmul(ps_x, wg1, xg_b1[:, sl], start=False, stop=True,
                                     skip_group_check=True)

                    s_sb = work.tile([128, NBLK], f32r, name=f"s_{i}", tag="s")
                    nc.scalar.activation(s_sb, ps_x, AF.Relu, bias=d_s, scale=1.0)

                    ps_p = pp_pool.tile([128, NBLK], f32, name=f"ps_p_{i}", tag="ps_p")
                    nc.tensor.matmul(ps_p, wpo_sb, s_sb, start=True, stop=True)

                    psi = work.tile([128, NBLK], f32, name=f"psi_{i}", tag="psi")
                    nc.scalar.activation(psi, ps_p, AF.Sigmoid, bias=d_p, scale=1.0)

                    nc.vector.tensor_mul(o_big[:, sl], xv, psi)

                nc.sync.dma_start(out=out_t[:, gsl], in_=o_big)

    nc.compile()
    return nc


def _get_program():
    if "nc" not in _CACHE:
        _CACHE["nc"] = _build_program()
    return _CACHE["nc"]


def kernel(input_g, input_x, Wg, bg, gamma_g, beta_g, mean_g, var_g,
           Wx, bx, gamma_x, beta_x, mean_x, var_x,
           Wp, bp, gamma_p, beta_p, mean_p, var_p):
    import sys
    if "/opt/trn_rl_repo" not in sys.path:
        sys.path.insert(0, "/opt/trn_rl_repo")
    from concourse import bass_utils

    nc = _get_program()

    # Fold BN (+conv bias) into weights/biases on host, in float64 for accuracy.
    f8 = np.float64
    A_g = (gamma_g.astype(f8) / np.sqrt(var_g.astype(f8) + EPS))
    C_g = beta_g.astype(f8) - mean_g.astype(f8) * A_g + bg.astype(f8) * A_g
    A_x = (gamma_x.astype(f8) / np.sqrt(var_x.astype(f8) + EPS))
    C_x = beta_x.astype(f8) - mean_x.astype(f8) * A_x + bx.astype(f8) * A_x
    A_p = (gamma_p.astype(f8) / np.sqrt(var_p.astype(f8) + EPS))[0]
    C_p = (beta_p.astype(f8) - mean_p.astype(f8) * A_p)[0]

    wg_eff = (Wg.astype(f8) * A_g[None, :]).astype(np.float32)
    wx_eff = (Wx.astype(f8) * A_x[None, :]).astype(np.float32)
    wpo = np.ascontiguousarray(
        np.repeat((Wp[:, 0].astype(f8) * A_p)[:, None], 128, axis=1)
    ).astype(np.float32)
    d_s = (C_g + C_x).astype(np.float32)
    d_x = C_x.astype(np.float32)
    d_p = np.full((128,), A_p * bp.astype(f8)[0] + C_p, dtype=np.float32)
    dvec = np.ascontiguousarray(np.stack([d_s, d_x, d_p], axis=1))

    in_maps = []
    for b in range(NCORES):
        xg_t = np.ascontiguousarray(input_g[b].reshape(NPIX, CIN).T)
        xx_t = np.ascontiguousarray(input_x[b].reshape(NPIX, CIN).T)
        in_maps.append(dict(xg=xg_t, xx=xx_t, wg=wg_eff, wx=wx_eff,
                            wpo=wpo, dvec=dvec))

    res = bass_utils.run_bass_kernel_spmd(nc, in_maps,
                                          core_ids=list(range(NCORES)))
    _CACHE["last_results"] = res

    out = np.empty((B, H, W, F), np.float32)
    for b in range(NCORES):
        out[b] = res.results[b]["out_t"].T.reshape(H, W, F)
    return out
